# revision 24
# baseline (speedup 1.0000x reference)
"""Trainium2 Bass kernel for nn_MeshLoss (chamfer-to-top-surface + fem MSE).

Computation (see reference):
  top  = network_mesh[:, :, :, -1, :]    -> B x 1024 "top surface" points (3D)
  dist2[b, m] = min_n || pc[b,:,m] - top[b,:,n] ||^2
  out = mean(dist2) + mean((network_mesh[...,:15,:] - fem_mesh[...,:15,:])**2)

Distribution: 8 cores = (B=4) x (M-half=2). Each core computes a partial
scalar; host sums the 8x3 partials.

Per-core algorithm:
  dot(p~, t~_n) = ||p - t_n||^2 - ||p||^2 with p~ = [p;1], t~ = [-2t; ||t||^2].
  Matmuls run in bf16 hi/lo form stacked to K=16 (hh+hl+lh+ll accumulated in
  fp32 PSUM -> ~fp32-accurate dots, single-pass bf16-speed matmuls).
  Per 128-point tile: bankA = dots vs tops 0:512, bankB = vs tops 512:1024.
  ACT copies bankB to SBUF, DVE tensor_tensor-min(A, Bcopy) -> pm (batch-2
  tiles per op), then one 3D tensor_reduce-min per 8 tiles -> per-point mins.
  ||p||^2 and fem MSE are ACT square+accumulate passes. Final partition
  reduction is a ones-vector matmul; host adds the 3 partials per core.
"""

import numpy as np
import ml_dtypes
from contextlib import ExitStack

B = 4
M = 16384
MSHARD = M // 2          # 8192 points per core
N = 1024                 # top surface points per batch
NH = N // 2              # 512 = bank width
MT = MSHARD // 128       # 64 m-tiles per core
CHAMFER_SCALE = 1.0 / float(B * M)          # 1/65536
FEM_SCALE = 1.0 / float(B * 3 * 32 * 15 * 32)   # 1/184320
WEIGHT = 1.0
TTB = 2                  # m-tiles per TT-min op (PSUM batch)
RDB = 4                  # m-tiles per 3D-reduce op

_NC_CACHE = {}


def _build_nc():
    import concourse.bacc as bacc
    import concourse.tile as tile
    import concourse.mybir as mybir

    f32 = mybir.dt.float32
    bf16 = mybir.dt.bfloat16
    ACTF = mybir.ActivationFunctionType
    ALU = mybir.AluOpType

    nc = bacc.Bacc("TRN2", target_bir_lowering=False, debug=False, num_devices=8)

    tops_d = nc.dram_tensor("tops", [3, N], f32, kind="ExternalInput").ap()
    topsT_d = nc.dram_tensor("topsT", [128, 24], f32, kind="ExternalInput").ap()
    pcsx_d = nc.dram_tensor("pcsx", [128, 256], f32, kind="ExternalInput").ap()
    nmb_d = nc.dram_tensor("nmb", [128, 180], f32, kind="ExternalInput").ap()
    femb_d = nc.dram_tensor("femb", [128, 180], f32, kind="ExternalInput").ap()
    ones_d = nc.dram_tensor("ones", [128, 1], f32, kind="ExternalInput").ap()
    out_d = nc.dram_tensor("out", [1, 3], f32, kind="ExternalOutput").ap()

    with tile.TileContext(nc) as tc, ExitStack() as ctx:
        const = ctx.enter_context(tc.tile_pool(name="const", bufs=1))
        sb = ctx.enter_context(tc.tile_pool(name="sb", bufs=3))
        pmpool = ctx.enter_context(tc.tile_pool(name="pmp", bufs=2))
        trees = ctx.enter_context(tc.tile_pool(name="trees", bufs=2))
        psum = ctx.enter_context(tc.tile_pool(name="psum", bufs=2, space="PSUM"))

        # ---------- loads (spread across the two DMA queues) ----------
        pcsx_sb = const.tile([128, 256], f32, tag="pcsx")
        nc.sync.dma_start(pcsx_sb[:], pcsx_d[:])
        top_sb = const.tile([3, N], f32, tag="top")
        nc.scalar.dma_start(top_sb[:], tops_d[:])
        topsT_sb = const.tile([128, 24], f32, tag="topsT")
        nc.scalar.dma_start(topsT_sb[:], topsT_d[:])
        ones_sb = const.tile([128, 1], f32, tag="ones")
        nc.scalar.dma_start(ones_sb[:], ones_d[:])
        nmb_sb = const.tile([128, 180], f32, tag="nmb")
        nc.sync.dma_start(nmb_sb[:], nmb_d[:])
        femb_sb = const.tile([128, 180], f32, tag="femb")
        nc.sync.dma_start(femb_sb[:], femb_d[:])

        engs = [nc.sync, nc.scalar]
        # lhsT16 rows per quarter q (partitions 32q..32q+15):
        #   [ph_c0, ph_c1, ph_c2, 1] x2, [pl_c0, pl_c1, pl_c2, 0] x2
        # pcsx rows per q: [pc_c0(8); pc_c1(8); pc_c2(8); ones(8)] so one
        # [32,256] hi/lo block maps to one [4,2048] dst block (flat order).
        phx = const.tile([128, 256], bf16, tag="phx")
        nc.vector.tensor_copy(phx[:], pcsx_sb[:])
        plx = const.tile([128, 256], bf16, tag="plx")
        nc.vector.tensor_sub(plx[:], pcsx_sb[:], phx[:])
        QW = MSHARD // 4                      # 2048 points per quarter
        p16 = const.tile([128, QW], bf16, tag="p16")
        di = 0
        for q in range(4):
            for dup in (0, 4):
                engs[di % 2].dma_start(p16[32 * q + dup:32 * q + dup + 4, :],
                                       phx[32 * q:32 * q + 32, :])
                di += 1
            for dup in (8, 12):
                engs[di % 2].dma_start(p16[32 * q + dup:32 * q + dup + 4, :],
                                       plx[32 * q:32 * q + 32, :])
                di += 1

        # ---------- prep: t4 = [-2t ; ||t||^2] (fp32) ----------
        # ||t||^2 from the transposed layout: [128, 8 pts, 3] -> reduce X
        sq2 = const.tile([128, 24], f32, tag="sq2")
        nc.vector.tensor_mul(sq2[:], topsT_sb[:], topsT_sb[:])
        normsq = const.tile([128, 8], f32, tag="normsq")
        nc.vector.tensor_reduce(normsq[:], sq2[:].rearrange("p (j c) -> p j c", c=3),
                                axis=mybir.AxisListType.X, op=ALU.add)
        t4 = const.tile([4, N], f32, tag="t4")
        nc.scalar.activation(t4[0:3, :], top_sb[:], ACTF.Copy, scale=-2.0)
        nc.scalar.dma_start(t4[3:4, :], normsq[:])

        # ---------- bf16 hi/lo decomposition ----------
        # rhs16 = [t4_hi; t4_lo; t4_hi; t4_lo]  (K=16)
        th = const.tile([4, N], bf16, tag="th")
        nc.vector.tensor_copy(th[:], t4[:])
        tl = const.tile([4, N], bf16, tag="tl")
        nc.vector.tensor_sub(tl[:], t4[:], th[:])
        # replicated to partition offsets 0/32/64/96 for 4-way row-group packing
        t16 = const.tile([128, N], bf16, tag="t16")
        for g in range(4):
            e = engs[g % 2]
            e.dma_start(t16[32 * g:32 * g + 4, :], th[:])
            e.dma_start(t16[32 * g + 4:32 * g + 8, :], tl[:])
            e.dma_start(t16[32 * g + 8:32 * g + 12, :], th[:])
            e.dma_start(t16[32 * g + 12:32 * g + 16, :], tl[:])

        mins = const.tile([128, MT], f32, tag="mins")
        cols = const.tile([128, 3], f32, tag="cols")
        nc.vector.memset(cols[:], 0.0)

        # ---------- ||p||^2 and fem MSE partials ----------
        p2j = pmpool.tile([128, 256], f32, tag="p2j")
        nc.scalar.activation(p2j[:], pcsx_sb[:], ACTF.Square,
                             scale=float(np.sqrt(CHAMFER_SCALE)),
                             accum_out=cols[:, 1:2])
        fdiff = pmpool.tile([128, 180], f32, tag="fdiff")
        nc.vector.tensor_sub(fdiff[:], nmb_sb[:], femb_sb[:])
        fj = pmpool.tile([128, 180], f32, tag="fj")
        nc.scalar.activation(fj[:], fdiff[:], ACTF.Square,
                             scale=float(np.sqrt(FEM_SCALE * WEIGHT)),
                             accum_out=cols[:, 2:3])

        # ---------- main chamfer loop ----------
        # PSUM slot [128, 2048] = [A_w|B_w|A_x|B_x] for m-tiles w, x taken
        # from two different quarters (row groups) so the 4 matmuls of
        # consecutive slots run concurrently in distinct 32-row PE groups.
        NLOC = MT // 4                      # 16 local tiles per quarter
        for l in range(NLOC):
            pmbig = pmpool.tile([128, 4 * NH], bf16, tag="pmbig")
            pm3 = pmbig[:].rearrange("p (g n) -> p g n", g=4)
            for half in range(2):           # quarters (0,1) then (2,3)
                ps = psum.tile([128, TTB * N], f32, tag="ps")
                for j in range(TTB):
                    q = 2 * half + j
                    g = 32 * q
                    cs = l * 128
                    nc.tensor.matmul(ps[:, j * N:j * N + NH],
                                     p16[g:g + 16, cs:cs + 128],
                                     t16[g:g + 16, 0:NH],
                                     start=True, stop=True,
                                     tile_position=(g, 0))
                    nc.tensor.matmul(ps[:, j * N + NH:(j + 1) * N],
                                     p16[g:g + 16, cs:cs + 128],
                                     t16[g:g + 16, NH:N],
                                     start=True, stop=True,
                                     tile_position=(g, 0))
                ps3 = ps[:].rearrange("p (g n) -> p g n", g=2 * TTB)
                # B banks are groups 1,3 (odd); A banks are 0,2
                if (2 * l + half) % 16 < 7:
                    # ACT-heavy: ACT casts both banks to bf16, DVE TT-min at 2x
                    bsa = sb.tile([128, TTB * NH], bf16, tag="bsa")
                    bsa3 = bsa[:].rearrange("p (g n) -> p g n", g=TTB)
                    nc.scalar.activation(bsa3[:, :, :], ps3[:, 0::2, :], ACTF.Copy)
                    bsb = sb.tile([128, TTB * NH], bf16, tag="bsb")
                    bsb3 = bsb[:].rearrange("p (g n) -> p g n", g=TTB)
                    nc.scalar.activation(bsb3[:, :, :], ps3[:, 1::2, :], ACTF.Copy)
                    nc.vector.tensor_tensor(pm3[:, 2 * half:2 * half + 2, :],
                                            bsa3[:, :, :], bsb3[:, :, :],
                                            op=ALU.min)
                else:
                    bs = sb.tile([128, TTB * NH], f32, tag="bs")
                    bs3 = bs[:].rearrange("p (g n) -> p g n", g=TTB)
                    nc.scalar.activation(bs3[:, :, :], ps3[:, 1::2, :], ACTF.Copy)
                    nc.vector.tensor_tensor(pm3[:, 2 * half:2 * half + 2, :],
                                            ps3[:, 0::2, :], bs3[:, :, :],
                                            op=ALU.min)
            # bf16 min-tree (TT-min runs 2x on packed bf16), then f32 reduce
            l1 = trees.tile([128, 4 * 256], bf16, tag="l1")
            l1_3 = l1[:].rearrange("p (g n) -> p g n", g=4)
            nc.vector.tensor_tensor(l1_3[:, :, :], pm3[:, :, 0:256],
                                    pm3[:, :, 256:512], op=ALU.min)
            l2 = trees.tile([128, 4 * 128], bf16, tag="l2")
            l2_3 = l2[:].rearrange("p (g n) -> p g n", g=4)
            nc.vector.tensor_tensor(l2_3[:, :, :], l1_3[:, :, 0:128],
                                    l1_3[:, :, 128:256], op=ALU.min)
            l3 = trees.tile([128, 4 * 64], bf16, tag="l3")
            l3_3 = l3[:].rearrange("p (g n) -> p g n", g=4)
            nc.vector.tensor_tensor(l3_3[:, :, :], l2_3[:, :, 0:64],
                                    l2_3[:, :, 64:128], op=ALU.min)
            nc.vector.tensor_reduce(mins[:, 4 * l:4 * l + 4],
                                    l3_3[:, :, :], axis=mybir.AxisListType.X,
                                    op=ALU.min)

        # ---------- final reduction ----------
        nc.vector.reduce_sum(cols[:, 0:1], mins[:], axis=mybir.AxisListType.X)
        nc.scalar.activation(cols[:, 0:1], cols[:, 0:1], ACTF.Copy,
                             scale=CHAMFER_SCALE)
        pf = psum.tile([1, 3], f32, tag="ps")
        nc.tensor.matmul(pf[:], ones_sb[:], cols[:], start=True, stop=True)
        out_sb = const.tile([1, 3], f32, tag="outsb")
        nc.scalar.activation(out_sb[:], pf[:], ACTF.Copy)
        nc.sync.dma_start(out_d[:], out_sb[:])

    nc.compile()
    return nc


def get_nc():
    if "nc" not in _NC_CACHE:
        _NC_CACHE["nc"] = _build_nc()
    return _NC_CACHE["nc"]


def shard_inputs(network_mesh, pc, fem_mesh):
    """Build the 8 per-core input maps (numpy slicing/layout only)."""
    network_mesh = np.ascontiguousarray(np.asarray(network_mesh, dtype=np.float32))
    pc = np.ascontiguousarray(np.asarray(pc, dtype=np.float32))
    fem_mesh = np.ascontiguousarray(np.asarray(fem_mesh, dtype=np.float32))
    ones_col = np.ones((128, 1), dtype=np.float32)
    in_maps = []
    for k in range(8):
        b, h = k // 2, k % 2
        tops = np.ascontiguousarray(network_mesh[b, :, :, 15, :].reshape(3, N))
        topsT = np.ascontiguousarray(tops.T.reshape(128, 24))
        pcs = pc[b, :, h * MSHARD:(h + 1) * MSHARD]
        pq = pcs.reshape(3, 4, 8, 256)
        ones8 = np.ones((8, 256), np.float32)
        pcsx = np.ascontiguousarray(np.concatenate(
            [np.concatenate([pq[0, q], pq[1, q], pq[2, q], ones8], axis=0)
             for q in range(4)], axis=0))
        nmb = np.ascontiguousarray(
            network_mesh[b, :, h * 16:(h + 1) * 16, 0:15, :].reshape(128, 180))
        femb = np.ascontiguousarray(
            fem_mesh[b, :, h * 16:(h + 1) * 16, 0:15, :].reshape(128, 180))
        in_maps.append({
            "tops": tops, "topsT": topsT, "pcsx": pcsx, "nmb": nmb,
            "femb": femb, "ones": ones_col,
        })
    return in_maps


def kernel(network_mesh, pc, fem_mesh):
    from concourse.bass_utils import run_bass_kernel_spmd

    nc = get_nc()
    in_maps = shard_inputs(network_mesh, pc, fem_mesh)
    res = run_bass_kernel_spmd(nc, in_maps, list(range(8)))
    total = np.float64(0.0)
    for r in res.results:
        total += np.float64(np.sum(np.asarray(r["out"], dtype=np.float64)))
        total -= 0.125   # ones-rows of pcsx in the ||p||^2 accumulation
    return np.float32(total)
